# revision 23
# baseline (speedup 1.0000x reference)
"""Blockwise 8x8 2D orthonormal DCT (Dct2d) for Trainium2, 8 NeuronCores.

Input  x: (64, 1, 1024, 1024) f32  ->  Output: (64, 64, 128, 128) f32
Data parallel over the batch dim: 8 samples per core.

Per-core algorithm (per 128-row strip of each 1024x1024 image), all matmuls
in fp16 (16-bit operands stream through the PE at 1 cycle/row vs 4 for
fp32; fp16's 10 mantissa bits keep max-rel error ~5e-4):
  cast (Pool):              xh = fp16(x strip)
  mm1 (per 128-col tile t): PSUM[w, (gh,i)] = Xh_t^T @ C,  C = I_16 (x) A^T
      (data tile is the *stationary* operand, so the transpose is fused)
  y1 copy (DVE):            one [128,1024] PSUM->SBUF copy, cast to fp16
  mm2 (per tile t):         PSUM[(gh,i), (j,gw16)] = Y1_t^T @ R,
      R[(g,l),(j,g)] = A[j,l]  (permuted block-diagonal)
  ot copy (Act):            strided [128,1024] PSUM->SBUF copy assembling
      [(gh,i), j, gw] so the HBM store has contiguous 512B runs.

Input DMAs issue on the SP queue, output DMAs on the Activation queue so
neither blocks the other. Each PSUM tile spans 2 banks (accumulation groups
stay per-bank); the single const load rides the Pool SWDGE path.

Steady state is DMA-bound at the cost model's 360 GB/s: 64 strips x
(512KB in + 512KB out) = 186.4us device-busy, >98% occupancy.
"""

from contextlib import ExitStack

import numpy as np

import concourse.bass as bass
import concourse.tile as tile
from concourse import bacc, mybir
from concourse.bass_utils import run_bass_kernel_spmd

N_CORES = 8
H = W = 1024
N_STRIPS = H // 128  # 8


def _dct_consts(A: np.ndarray) -> np.ndarray:
    A = np.asarray(A, np.float32)
    C = np.zeros((128, 128), np.float32)
    R = np.zeros((128, 128), np.float32)
    for g in range(16):
        C[g * 8 : (g + 1) * 8, g * 8 : (g + 1) * 8] = A.T
    for g in range(16):
        for l in range(8):
            for j in range(8):
                R[g * 8 + l, j * 16 + g] = A[j, l]
    # only C is loaded: R[:, j*16+g] == C[:, g*8+j], so mm2 streams a
    # strided column-permuted view of the same tile
    return C.astype(np.float16)


def _build(samples: int, CRmat: np.ndarray) -> bass.Bass:
    nc = bacc.Bacc(
        "TRN2", target_bir_lowering=False, debug=False, num_devices=N_CORES
    )
    f32 = mybir.dt.float32
    f16 = mybir.dt.float16
    x_ap = nc.dram_tensor("x", (samples, H, W), f32, kind="ExternalInput").ap()
    out_ap = nc.dram_tensor(
        "out", (samples, 64, H // 8, W // 8), f32, kind="ExternalOutput"
    ).ap()
    crd = nc.inline_tensor(CRmat, name="crmat").ap()

    with tile.TileContext(nc) as tc, ExitStack() as ctx:
        consts = ctx.enter_context(tc.tile_pool(name="consts", bufs=1))
        xpool = ctx.enter_context(tc.tile_pool(name="xs", bufs=6))
        xhpool = ctx.enter_context(tc.tile_pool(name="xh", bufs=4))
        y1pool = ctx.enter_context(tc.tile_pool(name="y1", bufs=4))
        opool = ctx.enter_context(tc.tile_pool(name="os", bufs=6))
        ps1 = ctx.enter_context(tc.tile_pool(name="ps1", bufs=2, space="PSUM"))
        ps2 = ctx.enter_context(tc.tile_pool(name="ps2", bufs=2, space="PSUM"))

        # strip-0 input DMA is issued first so the HBM read starts during
        # program warm-up; the consts ride the Pool SWDGE path in parallel
        xt0 = xpool.tile([128, 1024], f32)
        nc.sync.dma_start(xt0[:], x_ap[0, 0:128, :])
        crt = consts.tile([128, 128], f16)
        nc.gpsimd.dma_start(crt[:], crd[:])
        ct = crt[:]
        rt = crt.rearrange("p (g j) -> p j g", g=16)

        for s in range(samples):
            for st in range(N_STRIPS):
                if s == 0 and st == 0:
                    xt = xt0
                else:
                    xt = xpool.tile([128, 1024], f32)
                    nc.sync.dma_start(
                        xt[:], x_ap[s, st * 128 : (st + 1) * 128, :]
                    )

                # fp16 copy of the strip for the PE (Pool is SBUF-only)
                xh = xhpool.tile([128, 1024], f16)
                nc.gpsimd.tensor_copy(xh[:], xt[:])

                # columns t*128 + (gh*8+i): row-DCT'd, transposed tiles.
                # p1 spans 2 PSUM banks; accumulation groups stay per-bank.
                y1 = y1pool.tile([128, 1024], f16)
                p1 = ps1.tile([128, 1024], f32)
                for t in range(8):
                    nc.tensor.matmul(
                        p1[:, t * 128 : (t + 1) * 128],
                        lhsT=xh[:, t * 128 : (t + 1) * 128],
                        rhs=ct,
                        start=(t % 4 == 0),
                        stop=(t % 4 == 3),
                    )
                nc.vector.tensor_copy(y1[:], p1[:])

                # [p=(gh,i), j, gw]
                ot = opool.tile([128, 8, 128], f32)
                p2 = ps2.tile([128, 1024], f32)
                for t in range(8):
                    nc.tensor.matmul(
                        p2[:, t * 128 : (t + 1) * 128],
                        lhsT=y1[:, t * 128 : (t + 1) * 128],
                        rhs=rt,
                        start=(t % 4 == 0),
                        stop=(t % 4 == 3),
                    )
                # psum col (b, t4, j, g) -> ot[:, j, b*64 + t4*16 + g]
                src = p2.rearrange("p (b t j g) -> p b t j g", b=2, t=4, j=8)
                dst = ot.rearrange("p j (b t g) -> p b t j g", b=2, t=4)
                nc.scalar.copy(dst, src)

                dram_view = out_ap[s, :, st * 16 : (st + 1) * 16, :].rearrange(
                    "(i j) gh gw -> gh i j gw", i=8
                )
                nc.scalar.dma_start(dram_view, ot[:])

    nc.compile()
    return nc


_cache: dict = {}


def _get_program(samples: int, A: np.ndarray) -> bass.Bass:
    key = (samples, A.tobytes())
    if key not in _cache:
        _cache[key] = _build(samples, _dct_consts(A))
    return _cache[key]


def _run(x, A, **spmd_kwargs):
    x = np.ascontiguousarray(np.asarray(x, dtype=np.float32))
    A = np.asarray(A, dtype=np.float32)
    N = x.shape[0]
    spc = N // N_CORES  # samples per core
    nc = _get_program(spc, A)
    in_maps = [
        {"x": np.ascontiguousarray(x[i * spc : (i + 1) * spc, 0])}
        for i in range(N_CORES)
    ]
    res = run_bass_kernel_spmd(nc, in_maps, list(range(N_CORES)), **spmd_kwargs)
    out = np.concatenate(
        [res.results[i]["out"] for i in range(N_CORES)], axis=0
    )
    return out.astype(np.float32, copy=False), res


def kernel(x, A):
    out, _ = _run(x, A)
    return out
